# revision 1
# baseline (speedup 1.0000x reference)
"""Trainium2 Bass kernel for nn_Encoder_29661044146233 (gnn_message_passing).

Approach
--------
Both graph blocks are linear per-frame maps on the 88-dim channel vector
(channels = 22 joints x (3 pos + 1 offset)), so they fold into the conv
weights.  The three stride-2 k=4 temporal convs then compose into a single
22-tap stride-8 conv (88 -> 66 channels; the final [..., :3] slice is folded
into the output channels).  Only output frames t=0 and t=255 see boundary
(padding) effects; those two columns are computed with separately probed
15-tap edge weight sets.

Composite weights are obtained on the host by probing the (folded) linear
pipeline with impulses in float64 -- exact to fp32 rounding.

Device kernel (per core, batch 16 of 128):
  - input pre-marshalled on host to channel-major, phase-deinterleaved
    layout [16, 88, 8*258] (one leading + one trailing zero-pad column per
    phase) so every tap's rhs is one contiguous 256-column slice.
  - 22 accumulating fp32 matmuls (K=88, M=66, N=512 = 2 batch x 256 frames)
    per batch-pair into one PSUM bank; DVE adds bias while copying
    PSUM -> SBUF; edge columns overwritten from a small separately
    accumulated [66, 2, 16] edge result; contiguous stores.
"""

import os
import sys

for _p in ("/opt/trn_rl_repo", "/root/.axon_site/_ro/trn_rl_repo"):
    if os.path.isdir(_p) and _p not in sys.path:
        sys.path.append(_p)

import numpy as np

TOPOLOGY = [0, 0, 1, 2, 3, 4, 0, 6, 7, 8, 0, 10, 11, 12, 12, 14, 15, 16, 12, 18, 19, 20]
J = 22
POS, OFF = 3, 1
CIN = 88
COUT = 66
NTAP = 22
NEDGE = 15
B, F, T = 128, 2048, 256
NCORES = 8
BL = B // NCORES          # batch per core
BPAIR = BL // 2           # matmul batch pairs (N = 512)
PCOLS = 258               # per-phase columns incl. zero pads
XCOLS = 8 * PCOLS


# ---------------------------------------------------------------------------
# host-side weight composition (float64 impulse probing)
# ---------------------------------------------------------------------------

def _adj():
    a = np.zeros((J, J), np.float64)
    for i, p in enumerate(TOPOLOGY):
        if i:
            a[p, i] = 1.0
    return a


def _conv_np(z, w, b):
    Bn, Fn, C = z.shape
    zp = np.zeros((Bn, Fn + 2, C), z.dtype)
    zp[:, 1:Fn + 1] = z
    Fo = Fn // 2
    out = np.zeros((Bn, Fo, w.shape[0]), z.dtype)
    for k in range(4):
        out += zp[:, k:k + 2 * Fo:2] @ w[:, :, k].T
    return out + b


def _graph_mat(A, n2n_w, n2n_b, e2n_we, e2n_wn, e2n_b,
               n2e_wn, n2e_we, n2e_b, lin_w, lin_b):
    def apply(z):
        sh = z.shape[:-1]
        zz = z.reshape(-1, J, 4)
        node, edge = zz[..., :POS], zz[..., POS:]
        agg_n = np.einsum('ij,bjc->bic', A, node)
        agg_e = np.einsum('ij,bjc->bic', A, edge)
        f1 = agg_n @ n2n_w + n2n_b
        f2 = agg_e @ e2n_we + node @ e2n_wn + e2n_b
        new_edge = (np.einsum('ji,bjc->bic', A, node) @ n2e_wn
                    + edge @ n2e_we + n2e_b)
        h = np.concatenate([f1, f2], axis=-1) @ lin_w + lin_b
        return np.concatenate([h, new_edge], axis=-1).reshape(*sh, 88)

    g = apply(np.zeros((1, 88)))[0]
    G = apply(np.eye(88)) - g
    return G.T, g


def _compose(P):
    A = _adj()
    P64 = {k: np.asarray(v, np.float64) for k, v in P.items()}
    gnames = ('n2n_w', 'n2n_b', 'e2n_we', 'e2n_wn', 'e2n_b',
              'n2e_wn', 'n2e_we', 'n2e_b', 'lin_w', 'lin_b')
    G1, g1 = _graph_mat(A, *[P64['g1_' + s] for s in gnames])
    G2, g2 = _graph_mat(A, *[P64['g2_' + s] for s in gnames])
    keep = np.array([4 * j + c for j in range(J) for c in range(POS)])

    def pipeline(x88):
        y = _conv_np(x88, P64['conv1_w'], P64['conv1_b'])
        y = y @ G1.T + g1
        y = _conv_np(y, P64['conv2_w'], P64['conv2_b'])
        y = y @ G2.T + g2
        y = _conv_np(y, P64['conv3_w'], P64['conv3_b'])
        return y[..., keep]

    Fp = 256
    Tp = Fp // 8
    zb = pipeline(np.zeros((1, Fp, 88)))[0]
    bint, bl, br = zb[Tp // 2], zb[0], zb[Tp - 1]

    mid = Fp // 2
    probes = np.zeros((8 * 88, Fp, 88))
    for r in range(8):
        for ic in range(88):
            probes[r * 88 + ic, mid + r, ic] = 1.0
    resp = pipeline(probes) - zb
    wint = np.zeros((NTAP, COUT, CIN))
    for r in range(8):
        for t in range(Tp):
            m = (mid + r) - 8 * t + 7
            if 0 <= m < NTAP:
                wint[m] = resp[r * 88:(r + 1) * 88, t, :].T

    probes = np.zeros((NEDGE * 88, Fp, 88))
    for f in range(NEDGE):
        for ic in range(88):
            probes[f * 88 + ic, f, ic] = 1.0
    resp = pipeline(probes) - zb
    wl = np.stack([resp[f * 88:(f + 1) * 88, 0, :].T for f in range(NEDGE)])

    probes = np.zeros((NEDGE * 88, Fp, 88))
    for f in range(NEDGE):
        for ic in range(88):
            probes[f * 88 + ic, Fp - NEDGE + f, ic] = 1.0
    resp = pipeline(probes) - zb
    wr = np.stack([resp[f * 88:(f + 1) * 88, Tp - 1, :].T for f in range(NEDGE)])

    return dict(wint=wint, bint=bint, wl=wl, wr=wr, bl=bl, br=br)


def _tap_slice(m):
    # out[t] += W[m] @ x[8t + m - 7]  ->  (phase, col0) in the padded layout
    if m < 7:
        return m + 1, 0
    if m < 15:
        return m - 7, 1
    return m - 15, 2


# ---------------------------------------------------------------------------
# device program (built/compiled once, reused across calls)
# ---------------------------------------------------------------------------

_STATE = {}


def _build_device():
    import concourse.bass as bass  # noqa: F401
    import concourse.tile as tile
    from concourse import bacc, mybir

    f32 = mybir.dt.float32
    nc = bacc.Bacc("TRN2", target_bir_lowering=False, debug=False,
                   num_devices=NCORES)

    xph_d = nc.dram_tensor("xph", [BL, CIN, XCOLS], f32, kind="ExternalInput")
    xedge_d = nc.dram_tensor("xedge", [CIN, 2, 16, BL], f32, kind="ExternalInput")
    wint_d = nc.dram_tensor("wint", [CIN, NTAP, COUT], f32, kind="ExternalInput")
    wedge_d = nc.dram_tensor("wedge", [CIN, 2, 16, COUT], f32, kind="ExternalInput")
    bias_d = nc.dram_tensor("bias", [COUT, 1], f32, kind="ExternalInput")
    out_d = nc.dram_tensor("out", [COUT, BL, T], f32, kind="ExternalOutput")

    with tile.TileContext(nc) as tc:
        with (
            tc.tile_pool(name="consts", bufs=1) as consts,
            tc.tile_pool(name="xp", bufs=BPAIR) as xpool,
            tc.tile_pool(name="ps", bufs=4, space="PSUM") as pspool,
            tc.tile_pool(name="eps", bufs=1, space="PSUM") as epspool,
            tc.tile_pool(name="ob", bufs=3) as opool,
        ):
            w_sb = consts.tile([CIN, NTAP, COUT], f32)
            nc.sync.dma_start(out=w_sb[:], in_=wint_d[:])
            we_sb = consts.tile([CIN, 2, 16, COUT], f32)
            nc.sync.dma_start(out=we_sb[:], in_=wedge_d[:])
            xe_sb = consts.tile([CIN, 2, 16, BL], f32)
            nc.sync.dma_start(out=xe_sb[:], in_=xedge_d[:])
            b_sb = consts.tile([COUT, 1], f32)
            nc.sync.dma_start(out=b_sb[:], in_=bias_d[:])

            # edge columns: one accumulation group over both sides
            eps = epspool.tile([COUT, 2, BL], f32)
            for side in range(2):
                for e in range(16):
                    nc.tensor.matmul(
                        eps[:, side, :],
                        lhsT=we_sb[:, side, e, :],
                        rhs=xe_sb[:, side, e, :],
                        start=(side == 0 and e == 0),
                        stop=(side == 1 and e == 15),
                    )
            eb = consts.tile([COUT, 2, BL], f32)
            nc.vector.tensor_scalar_add(eb[:], eps[:], b_sb[:])

            # main conv: batch pairs, N = 512 per matmul
            for g in range(BPAIR):
                xt = xpool.tile([CIN, 2, XCOLS], f32)
                nc.sync.dma_start(out=xt[:], in_=xph_d[2 * g:2 * g + 2].rearrange("b c x -> c b x"))
                ps = pspool.tile([COUT, 2, T], f32)
                for m in range(NTAP):
                    p, c0 = _tap_slice(m)
                    col = p * PCOLS + c0
                    nc.tensor.matmul(
                        ps[:],
                        lhsT=w_sb[:, m, :],
                        rhs=xt[:, :, col:col + 256],
                        start=(m == 0),
                        stop=(m == NTAP - 1),
                    )
                ob = opool.tile([COUT, 2, T], f32)
                nc.vector.tensor_scalar_add(ob[:], ps[:], b_sb[:])
                for j in range(2):
                    b_idx = 2 * g + j
                    nc.vector.tensor_copy(out=ob[:, j, 0:1], in_=eb[:, 0, b_idx:b_idx + 1])
                    nc.vector.tensor_copy(out=ob[:, j, T - 1:T], in_=eb[:, 1, b_idx:b_idx + 1])
                nc.sync.dma_start(out=out_d[:, 2 * g:2 * g + 2, :], in_=ob[:])

    nc.compile()
    return nc


def _get_state():
    if "nc" not in _STATE:
        _STATE["nc"] = _build_device()
    return _STATE["nc"]


# ---------------------------------------------------------------------------
# entry point
# ---------------------------------------------------------------------------

def kernel(**inputs):
    from concourse.bass_utils import run_bass_kernel_spmd

    P = {k: np.asarray(v) for k, v in inputs.items()}
    inp = P.pop("input").astype(np.float32, copy=False)
    off = P.pop("offset").astype(np.float32, copy=False)

    C = _compose(P)

    x88T = np.ascontiguousarray(
        np.concatenate([inp, off], -1).reshape(B, F, CIN).transpose(0, 2, 1))

    xph = np.zeros((B, CIN, 8, PCOLS), np.float32)
    xph[:, :, :, 1:257] = x88T.reshape(B, CIN, T, 8).transpose(0, 1, 3, 2)
    xph = xph.reshape(B, CIN, XCOLS)

    xedge = np.zeros((B, CIN, 2, 16), np.float32)
    xedge[:, :, 0, :NEDGE] = x88T[:, :, :NEDGE]
    xedge[:, :, 1, :NEDGE] = x88T[:, :, F - NEDGE:]
    xedge[:, 0, :, 15] = 1.0

    wint = np.ascontiguousarray(
        C["wint"].transpose(2, 0, 1)).astype(np.float32)        # [88, 22, 66]
    wedge = np.zeros((CIN, 2, 16, COUT), np.float32)
    wedge[:, 0, :NEDGE, :] = C["wl"].transpose(2, 0, 1)
    wedge[:, 1, :NEDGE, :] = C["wr"].transpose(2, 0, 1)
    wedge[0, 0, 15, :] = C["bl"] - C["bint"]
    wedge[0, 1, 15, :] = C["br"] - C["bint"]
    bias = C["bint"].astype(np.float32).reshape(COUT, 1)

    in_maps = []
    for c in range(NCORES):
        s = slice(c * BL, (c + 1) * BL)
        in_maps.append({
            "xph": xph[s],
            "xedge": np.ascontiguousarray(xedge[s].transpose(1, 2, 3, 0)),
            "wint": wint,
            "wedge": wedge,
            "bias": bias,
        })

    nc = _get_state()
    res = run_bass_kernel_spmd(nc, in_maps, core_ids=list(range(NCORES)))

    out = np.empty((B, T, J, POS), np.float32)
    for c in range(NCORES):
        o = res.results[c]["out"]                                # [66, BL, 256]
        out[c * BL:(c + 1) * BL] = o.transpose(1, 2, 0).reshape(BL, T, J, POS)
    return out


# revision 4
# speedup vs baseline: 2.9659x; 2.9659x over previous
"""Trainium2 Bass kernel for nn_Encoder_29661044146233 (gnn_message_passing).

Approach
--------
Both graph blocks are linear per-frame maps on the 88-dim channel vector
(channels = 22 joints x (3 pos + 1 offset)), so they fold into the conv
weights.  The three stride-2 k=4 temporal convs then compose into a single
22-tap stride-8 conv (88 -> 66 channels; the final [..., :3] slice is folded
into the output channels).  Only output frames t=0 and t=255 see boundary
(padding) effects; those two columns are computed with separately probed
15-tap edge weight sets.

Composite weights are obtained on the host by probing the (folded) linear
pipeline with impulses in float64 -- exact to fp32 rounding.

Device kernel (per core, batch 16 of 128):
  - input pre-marshalled on host to channel-major, phase-deinterleaved
    layout [16, 88, 8*258] (one leading + one trailing zero-pad column per
    phase) so every tap's rhs is one contiguous 256-column slice.
  - 22 accumulating fp32 matmuls (K=88, M=66, N=512 = 2 batch x 256 frames)
    per batch-pair into one PSUM bank; DVE adds bias while copying
    PSUM -> SBUF; edge columns overwritten from a small separately
    accumulated [66, 2, 16] edge result; contiguous stores.
"""

import os
import sys

for _p in ("/opt/trn_rl_repo", "/root/.axon_site/_ro/trn_rl_repo"):
    if os.path.isdir(_p) and _p not in sys.path:
        sys.path.append(_p)

import numpy as np

TOPOLOGY = [0, 0, 1, 2, 3, 4, 0, 6, 7, 8, 0, 10, 11, 12, 12, 14, 15, 16, 12, 18, 19, 20]
J = 22
POS, OFF = 3, 1
CIN = 88
COUT = 66
NTAP = 22
NEDGE = 15
B, F, T = 128, 2048, 256
NCORES = 8
BL = B // NCORES          # batch per core
BPAIR = BL // 2           # matmul batch pairs (N = 512)
PCOLS = 258               # per-phase columns incl. zero pads
XCOLS = 8 * PCOLS


# ---------------------------------------------------------------------------
# host-side weight composition (float64 impulse probing)
# ---------------------------------------------------------------------------

def _adj():
    a = np.zeros((J, J), np.float64)
    for i, p in enumerate(TOPOLOGY):
        if i:
            a[p, i] = 1.0
    return a


def _conv_np(z, w, b):
    Bn, Fn, C = z.shape
    zp = np.zeros((Bn, Fn + 2, C), z.dtype)
    zp[:, 1:Fn + 1] = z
    Fo = Fn // 2
    out = np.zeros((Bn, Fo, w.shape[0]), z.dtype)
    for k in range(4):
        out += zp[:, k:k + 2 * Fo:2] @ w[:, :, k].T
    return out + b


def _graph_mat(A, n2n_w, n2n_b, e2n_we, e2n_wn, e2n_b,
               n2e_wn, n2e_we, n2e_b, lin_w, lin_b):
    def apply(z):
        sh = z.shape[:-1]
        zz = z.reshape(-1, J, 4)
        node, edge = zz[..., :POS], zz[..., POS:]
        agg_n = np.einsum('ij,bjc->bic', A, node)
        agg_e = np.einsum('ij,bjc->bic', A, edge)
        f1 = agg_n @ n2n_w + n2n_b
        f2 = agg_e @ e2n_we + node @ e2n_wn + e2n_b
        new_edge = (np.einsum('ji,bjc->bic', A, node) @ n2e_wn
                    + edge @ n2e_we + n2e_b)
        h = np.concatenate([f1, f2], axis=-1) @ lin_w + lin_b
        return np.concatenate([h, new_edge], axis=-1).reshape(*sh, 88)

    g = apply(np.zeros((1, 88)))[0]
    G = apply(np.eye(88)) - g
    return G.T, g


def _compose(P):
    A = _adj()
    P64 = {k: np.asarray(v, np.float64) for k, v in P.items()}
    gnames = ('n2n_w', 'n2n_b', 'e2n_we', 'e2n_wn', 'e2n_b',
              'n2e_wn', 'n2e_we', 'n2e_b', 'lin_w', 'lin_b')
    G1, g1 = _graph_mat(A, *[P64['g1_' + s] for s in gnames])
    G2, g2 = _graph_mat(A, *[P64['g2_' + s] for s in gnames])
    keep = np.array([4 * j + c for j in range(J) for c in range(POS)])

    def pipeline(x88):
        y = _conv_np(x88, P64['conv1_w'], P64['conv1_b'])
        y = y @ G1.T + g1
        y = _conv_np(y, P64['conv2_w'], P64['conv2_b'])
        y = y @ G2.T + g2
        y = _conv_np(y, P64['conv3_w'], P64['conv3_b'])
        return y[..., keep]

    Fp = 256
    Tp = Fp // 8
    zb = pipeline(np.zeros((1, Fp, 88)))[0]
    bint, bl, br = zb[Tp // 2], zb[0], zb[Tp - 1]

    mid = Fp // 2
    probes = np.zeros((8 * 88, Fp, 88))
    for r in range(8):
        for ic in range(88):
            probes[r * 88 + ic, mid + r, ic] = 1.0
    resp = pipeline(probes) - zb
    wint = np.zeros((NTAP, COUT, CIN))
    for r in range(8):
        for t in range(Tp):
            m = (mid + r) - 8 * t + 7
            if 0 <= m < NTAP:
                wint[m] = resp[r * 88:(r + 1) * 88, t, :].T

    probes = np.zeros((NEDGE * 88, Fp, 88))
    for f in range(NEDGE):
        for ic in range(88):
            probes[f * 88 + ic, f, ic] = 1.0
    resp = pipeline(probes) - zb
    wl = np.stack([resp[f * 88:(f + 1) * 88, 0, :].T for f in range(NEDGE)])

    probes = np.zeros((NEDGE * 88, Fp, 88))
    for f in range(NEDGE):
        for ic in range(88):
            probes[f * 88 + ic, Fp - NEDGE + f, ic] = 1.0
    resp = pipeline(probes) - zb
    wr = np.stack([resp[f * 88:(f + 1) * 88, Tp - 1, :].T for f in range(NEDGE)])

    return dict(wint=wint, bint=bint, wl=wl, wr=wr, bl=bl, br=br)


def _tap_slice(m):
    # out[t] += W[m] @ x[8t + m - 7]  ->  (phase, col0) in the padded layout
    if m < 7:
        return m + 1, 0
    if m < 15:
        return m - 7, 1
    return m - 15, 2


# ---------------------------------------------------------------------------
# device program (built/compiled once, reused across calls)
# ---------------------------------------------------------------------------

_STATE = {}


def _build_device():
    import concourse.bass as bass  # noqa: F401
    import concourse.tile as tile
    from concourse import bacc, mybir

    f32 = mybir.dt.float32
    # float32r: single-pass fp32 matmul (1 col/cycle vs 4 for exact fp32).
    # Measured on HW for this exact shape: rel err ~1.5e-4 vs float64.
    # Exact fp32 (4x slower on PE) available by flipping this to f32.
    mmdt = mybir.dt.float32r
    nc = bacc.Bacc("TRN2", target_bir_lowering=False, debug=False,
                   num_devices=NCORES)

    xph_d = nc.dram_tensor("xph", [BL, CIN, XCOLS], mmdt, kind="ExternalInput")
    xedge_d = nc.dram_tensor("xedge", [CIN, 2, 16, BL], mmdt, kind="ExternalInput")
    wint_d = nc.dram_tensor("wint", [CIN, NTAP, COUT], mmdt, kind="ExternalInput")
    wedge_d = nc.dram_tensor("wedge", [CIN, 2, 16, COUT], mmdt, kind="ExternalInput")
    bias_d = nc.dram_tensor("bias", [COUT, 1], f32, kind="ExternalInput")
    out_d = nc.dram_tensor("out", [COUT, BL, T], f32, kind="ExternalOutput")

    with tile.TileContext(nc) as tc:
        with (
            tc.tile_pool(name="consts", bufs=1) as consts,
            tc.tile_pool(name="xp", bufs=BPAIR) as xpool,
            tc.tile_pool(name="ps", bufs=4, space="PSUM") as pspool,
            tc.tile_pool(name="eps", bufs=1, space="PSUM") as epspool,
            tc.tile_pool(name="ob", bufs=3) as opool,
        ):
            w_sb = consts.tile([CIN, NTAP, COUT], mmdt)
            nc.sync.dma_start(out=w_sb[:], in_=wint_d[:])
            we_sb = consts.tile([CIN, 2, 16, COUT], mmdt)
            nc.sync.dma_start(out=we_sb[:], in_=wedge_d[:])
            xe_sb = consts.tile([CIN, 2, 16, BL], mmdt)
            nc.sync.dma_start(out=xe_sb[:], in_=xedge_d[:])
            b_sb = consts.tile([COUT, 1], f32)
            nc.sync.dma_start(out=b_sb[:], in_=bias_d[:])

            # edge columns: one accumulation group over both sides
            eps = epspool.tile([COUT, 2, BL], f32)
            for side in range(2):
                for e in range(16):
                    nc.tensor.matmul(
                        eps[:, side, :],
                        lhsT=we_sb[:, side, e, :],
                        rhs=xe_sb[:, side, e, :],
                        start=(side == 0 and e == 0),
                        stop=(side == 1 and e == 15),
                    )
            eb = consts.tile([COUT, 2, BL], f32)
            nc.vector.tensor_scalar_add(eb[:], eps[:], b_sb[:])

            # main conv: batch pairs, N = 512 per matmul
            for g in range(BPAIR):
                xt = xpool.tile([CIN, 2, XCOLS], mmdt)
                nc.sync.dma_start(out=xt[:], in_=xph_d[2 * g:2 * g + 2].rearrange("b c x -> c b x"))
                ps = pspool.tile([COUT, 2, T], f32)
                for m in range(NTAP):
                    p, c0 = _tap_slice(m)
                    col = p * PCOLS + c0
                    nc.tensor.matmul(
                        ps[:],
                        lhsT=w_sb[:, m, :],
                        rhs=xt[:, :, col:col + 256],
                        start=(m == 0),
                        stop=(m == NTAP - 1),
                    )
                ob = opool.tile([COUT, 2, T], f32)
                nc.vector.tensor_scalar_add(ob[:], ps[:], b_sb[:])
                for j in range(2):
                    b_idx = 2 * g + j
                    nc.vector.tensor_copy(out=ob[:, j, 0:1], in_=eb[:, 0, b_idx:b_idx + 1])
                    nc.vector.tensor_copy(out=ob[:, j, T - 1:T], in_=eb[:, 1, b_idx:b_idx + 1])
                nc.sync.dma_start(out=out_d[:, 2 * g:2 * g + 2, :], in_=ob[:])

    nc.compile()
    return nc


def _get_state():
    if "nc" not in _STATE:
        _STATE["nc"] = _build_device()
    return _STATE["nc"]


# ---------------------------------------------------------------------------
# entry point
# ---------------------------------------------------------------------------

def kernel(**inputs):
    from concourse.bass_utils import run_bass_kernel_spmd

    P = {k: np.asarray(v) for k, v in inputs.items()}
    inp = P.pop("input").astype(np.float32, copy=False)
    off = P.pop("offset").astype(np.float32, copy=False)

    C = _compose(P)

    x88T = np.ascontiguousarray(
        np.concatenate([inp, off], -1).reshape(B, F, CIN).transpose(0, 2, 1))

    xph = np.zeros((B, CIN, 8, PCOLS), np.float32)
    xph[:, :, :, 1:257] = x88T.reshape(B, CIN, T, 8).transpose(0, 1, 3, 2)
    xph = xph.reshape(B, CIN, XCOLS)

    xedge = np.zeros((B, CIN, 2, 16), np.float32)
    xedge[:, :, 0, :NEDGE] = x88T[:, :, :NEDGE]
    xedge[:, :, 1, :NEDGE] = x88T[:, :, F - NEDGE:]
    xedge[:, 0, :, 15] = 1.0

    wint = np.ascontiguousarray(
        C["wint"].transpose(2, 0, 1)).astype(np.float32)        # [88, 22, 66]
    wedge = np.zeros((CIN, 2, 16, COUT), np.float32)
    wedge[:, 0, :NEDGE, :] = C["wl"].transpose(2, 0, 1)
    wedge[:, 1, :NEDGE, :] = C["wr"].transpose(2, 0, 1)
    wedge[0, 0, 15, :] = C["bl"] - C["bint"]
    wedge[0, 1, 15, :] = C["br"] - C["bint"]
    bias = C["bint"].astype(np.float32).reshape(COUT, 1)

    in_maps = []
    for c in range(NCORES):
        s = slice(c * BL, (c + 1) * BL)
        in_maps.append({
            "xph": xph[s],
            "xedge": np.ascontiguousarray(xedge[s].transpose(1, 2, 3, 0)),
            "wint": wint,
            "wedge": wedge,
            "bias": bias,
        })

    nc = _get_state()
    res = run_bass_kernel_spmd(nc, in_maps, core_ids=list(range(NCORES)))

    out = np.empty((B, T, J, POS), np.float32)
    for c in range(NCORES):
        o = res.results[c]["out"]                                # [66, BL, 256]
        out[c * BL:(c + 1) * BL] = o.transpose(1, 2, 0).reshape(BL, T, J, POS)
    return out
